# revision 18
# baseline (speedup 1.0000x reference)
"""Bass/Trainium2 kernel for BiasMultiHeadAttention.

Reference computation (B=2, N=512, D=256, H=8, hd=32):
    q = (X @ Wq + bq), k = (X @ Wk + bk), v = (X @ Wv + bv)      [B,N,H,hd]
    bias = (relu(E @ We1 + be1) @ We2 + be2)[..., 0]             [B,N,N]
    scores = einsum(q,k)/sqrt(hd) + bias[:,None]                 [B,H,N,N]
    out = (softmax(scores) @ v) @ Wo + bo                        [B,N,D]

Sharding: pure data parallel over (batch, query-block). Core c handles
batch c//4 and query rows (c%4)*128..+128. Each core reads only its
[128, 512, 256] slice of E (64 MiB — the dominant HBM traffic), computes
its slice of the edge-MLP bias, q for its rows, full k/v (cheap), and
produces its [128, 256] slice of the output. No collectives.

Exact simplifications (softmax shift invariance over the key axis):
  - bk drops out (q·bk is constant over j)
  - be2 drops out (constant over j)
  - bv/bo folded in as additive vectors at the v / output copies
"""

import os

import numpy as np

import concourse.bass as bass
import concourse.mybir as mybir
import concourse.tile as tile
from concourse import bacc
from concourse.bass_utils import run_bass_kernel_spmd
from concourse.masks import make_identity

F32 = mybir.dt.float32
BF16 = mybir.dt.bfloat16

B, N, D = 2, 512, 256
H, HD = 8, 32
NCORES = 8
IB = 128  # query rows per core
SCALE = 1.0 / np.sqrt(HD)


def build(compile=True):
    nc = bacc.Bacc("TRN2", target_bir_lowering=False)

    e_d = nc.declare_dram_parameter("E", [IB, N, D], F32, isOutput=False)
    xt_d = nc.declare_dram_parameter("XT", [D, N], F32, isOutput=False)
    xtq_d = nc.declare_dram_parameter("XTQ", [D, IB], F32, isOutput=False)
    w_d = {
        name: nc.declare_dram_parameter(name, [D, D], F32, isOutput=False)
        for name in ("Wq", "Wk", "Wv", "Wo", "We1")
    }
    we2_d = nc.declare_dram_parameter("We2", [D, 1], F32, isOutput=False)
    b_d = {
        name: nc.declare_dram_parameter(name, [D], F32, isOutput=False)
        for name in ("bq", "be1", "bv", "bo")
    }
    out_d = nc.declare_dram_parameter("OUT_T", [D, IB], F32, isOutput=True)

    def mm(out, lhsT, rhs, start, stop, tile_position=None):
        nc.tensor.matmul(
            out, lhsT, rhs, start=start, stop=stop, tile_position=tile_position
        )

    with tile.TileContext(nc) as tc, tc.tile_pool(name="consts", bufs=1) as consts:
        ident = consts.tile([128, 128], F32)
        make_identity(nc, ident)

        # Weights [D, D] -> sbuf [128, (a=2) * (n=256)]; lhsT chunk (k, m)
        # = rows k*128..+128, cols m*128..+128 of W.
        w_sb = {}
        for name, d in w_d.items():
            t = consts.tile([128, 2, D], F32, name=f"w_{name}")
            nc.sync.dma_start(out=t, in_=d.rearrange("(a p) n -> p a n", p=128))
            w_sb[name] = t

        def wchunk(name, k, m):
            return w_sb[name][:, k, m * 128 : m * 128 + 128]

        # bf16 copies of the edge-MLP weights (the E path runs in bf16;
        # SWDGE DMA does the fp32->bf16 cast on load)
        we1_bf = consts.tile([128, 2, D], BF16)
        nc.gpsimd.dma_start(
            out=we1_bf, in_=w_d["We1"].rearrange("(a p) n -> p a n", p=128)
        )
        we2_bf = consts.tile([128, 2], BF16)
        nc.gpsimd.dma_start(
            out=we2_bf, in_=we2_d.rearrange("(a p) n -> p a n", p=128).rearrange(
                "p a n -> p (a n)"
            )
        )

        b_sb = {}
        for name, d in b_d.items():
            t = consts.tile([128, 2], F32, name=f"b_{name}")
            nc.sync.dma_start(out=t, in_=d.rearrange("(a p) -> p a", p=128))
            b_sb[name] = t

        # pre-scaled q bias: (x@Wq)*s + bq*s
        bqs = consts.tile([128, 2], F32)
        nc.scalar.mul(bqs, b_sb["bq"], SCALE)

        xt_sb = consts.tile([128, 2, N], F32)
        nc.sync.dma_start(out=xt_sb, in_=xt_d.rearrange("(a p) n -> p a n", p=128))
        xtq_sb = consts.tile([128, 2, IB], F32)
        nc.sync.dma_start(out=xtq_sb, in_=xtq_d.rearrange("(a p) n -> p a n", p=128))

        # persistent intermediates
        bias_sb = consts.tile([128, N], F32)  # edge-MLP attention bias [i, j]
        kt_sb = consts.tile([128, 2, N], F32)  # k^T [d, j]
        vn_sb = consts.tile([128, 4, D], F32)  # v natural [j, d] (4 j-chunks)
        qt_sb = consts.tile([128, 2, IB], F32)  # q^T [d, i] (pre-scaled)
        yt_sb = consts.tile([128, 2, IB], F32)  # attn-out^T [d, i]
        outt_sb = consts.tile([128, 2, IB], F32)

        # ---------------- Phase 0: q/k/v projections ----------------
        with (
            tc.tile_pool(name="p0psum", bufs=2, space="PSUM") as p0psum,
            tc.tile_pool(name="p0tr", bufs=2, space="PSUM") as p0tr,
            tc.tile_pool(name="p0sb", bufs=2) as p0sb,
        ):
            # k^T = Wk^T @ X^T   (bk dropped: softmax-invariant)
            for cc in range(2):
                ps = p0psum.tile([128, N], F32)
                for k in range(2):
                    mm(ps, wchunk("Wk", k, cc), xt_sb[:, k, :], k == 0, k == 1)
                nc.scalar.copy(kt_sb[:, cc, :], ps)

            # v^T (+bv) then PE-transpose to natural [j, d]
            for cc in range(2):
                ps = p0psum.tile([128, N], F32)
                for k in range(2):
                    mm(ps, wchunk("Wv", k, cc), xt_sb[:, k, :], k == 0, k == 1)
                vt_tmp = p0sb.tile([128, N], F32)
                nc.scalar.activation(
                    vt_tmp, ps, mybir.ActivationFunctionType.Identity,
                    bias=b_sb["bv"][:, cc : cc + 1],
                )
                pt = p0tr.tile([128, N], F32)
                for jc in range(4):
                    nc.tensor.transpose(
                        pt[:, jc * 128 : (jc + 1) * 128],
                        vt_tmp[:, jc * 128 : (jc + 1) * 128],
                        ident,
                    )
                for jc in range(4):
                    dst = vn_sb[:, jc, cc * 128 : (cc + 1) * 128]
                    src = pt[:, jc * 128 : (jc + 1) * 128]
                    if jc % 2 == 0:
                        nc.scalar.copy(dst, src)
                    else:
                        nc.vector.tensor_copy(dst, src)

            # q^T for this core's rows, pre-scaled by 1/sqrt(hd)
            for cc in range(2):
                ps = p0psum.tile([128, IB], F32)
                for k in range(2):
                    mm(ps, wchunk("Wq", k, cc), xtq_sb[:, k, :], k == 0, k == 1)
                nc.scalar.activation(
                    qt_sb[:, cc, :], ps, mybir.ActivationFunctionType.Identity,
                    bias=bqs[:, cc : cc + 1], scale=SCALE,
                )

        # ---------------- Phase 1: edge MLP over E ----------------
        with (
            tc.tile_pool(name="e_pool", bufs=3) as e_pool,
            tc.tile_pool(name="eT_pool", bufs=2) as eT_pool,
            tc.tile_pool(name="h_pool", bufs=2) as h_pool,
            tc.tile_pool(name="stage_pool", bufs=4) as stage_pool,
            tc.tile_pool(name="ptr", bufs=4, space="PSUM") as ptr_pool,
            tc.tile_pool(name="ph", bufs=2, space="PSUM") as ph_pool,
            tc.tile_pool(name="pb", bufs=2, space="PSUM") as pb_pool,
        ):
            for i in range(IB):
                e_nat = e_pool.tile([128, 4, D], F32)
                nc.sync.dma_start(
                    out=e_nat, in_=e_d[i].rearrange("(t p) d -> p t d", p=128)
                )
                et = eT_pool.tile([128, 2, N], BF16)
                for cc in range(2):
                    pt = ptr_pool.tile([128, N], F32)
                    for t in range(4):
                        nc.tensor.transpose(
                            pt[:, t * 128 : (t + 1) * 128],
                            e_nat[:, t, cc * 128 : (cc + 1) * 128],
                            ident,
                        )
                    if cc == 0:
                        nc.scalar.copy(et[:, cc, :], pt)
                    else:
                        nc.vector.tensor_copy(et[:, cc, :], pt)

                h = h_pool.tile([128, 2, N], BF16)
                for o in range(2):
                    ph = ph_pool.tile([128, N], F32)
                    for k in range(2):
                        mm(ph, we1_bf[:, k, o * 128 : o * 128 + 128], et[:, k, :],
                           k == 0, k == 1)
                    if o == 0:
                        nc.scalar.activation(
                            h[:, o, :], ph, mybir.ActivationFunctionType.Relu,
                            bias=b_sb["be1"][:, o : o + 1],
                        )
                    else:
                        nc.vector.tensor_scalar(
                            h[:, o, :], ph, b_sb["be1"][:, o : o + 1], 0.0,
                            mybir.AluOpType.add, mybir.AluOpType.max,
                        )

                pb = pb_pool.tile([1, N], F32)
                for o in range(2):
                    mm(pb, we2_bf[:, o : o + 1], h[:, o, :], o == 0, o == 1)
                # engines cannot write arbitrary partition rows; stage at
                # partition 0 and let DMA (which can remap partitions) place
                # the row into bias_sb[i].
                brow = stage_pool.tile([1, N], F32)
                nc.vector.tensor_copy(brow, pb)
                nc.sync.dma_start(out=bias_sb[i : i + 1, :], in_=brow)

        # ---------------- Phase 2: attention ----------------
        with (
            tc.tile_pool(name="ps_s", bufs=2, space="PSUM") as ps_s,
            tc.tile_pool(name="ps_at", bufs=2, space="PSUM") as ps_at,
            tc.tile_pool(name="ps_y", bufs=2, space="PSUM") as ps_y,
            tc.tile_pool(name="sm_sb", bufs=4) as sm_sb,
            tc.tile_pool(name="at_sb", bufs=2) as at_pool,
            tc.tile_pool(name="stat", bufs=12) as stat,
        ):
            for g in range(2):
                py = ps_y.tile([128, IB], F32)
                for hh in range(4):
                    ps = ps_s.tile([128, N], F32)
                    mm(
                        ps,
                        qt_sb[hh * 32 : (hh + 1) * 32, g, :],
                        kt_sb[hh * 32 : (hh + 1) * 32, g, :],
                        True, True, tile_position=(hh * 32, 0),
                    )
                    s_t = sm_sb.tile([128, N], F32)
                    nc.vector.tensor_tensor(s_t, ps, bias_sb, mybir.AluOpType.add)
                    nmax = stat.tile([128, 1], F32)
                    nc.vector.reduce_max(
                        nmax, s_t, axis=mybir.AxisListType.X, negate=True
                    )
                    p_t = sm_sb.tile([128, N], F32)
                    ssum = stat.tile([128, 1], F32)
                    nc.scalar.activation(
                        p_t, s_t, mybir.ActivationFunctionType.Exp,
                        bias=nmax, accum_out=ssum,
                    )
                    rinv = stat.tile([128, 1], F32)
                    nc.vector.reciprocal(rinv, ssum)
                    nc.vector.tensor_scalar_mul(p_t, p_t, rinv)

                    pat = ps_at.tile([128, N], F32)
                    for t in range(4):
                        nc.tensor.transpose(
                            pat[:, t * 128 : (t + 1) * 128],
                            p_t[:, t * 128 : (t + 1) * 128],
                            ident,
                        )
                    at_t = at_pool.tile([128, 4, IB], F32)
                    if hh % 2 == 0:
                        nc.scalar.copy(at_t.rearrange("p a b -> p (a b)"), pat)
                    else:
                        nc.vector.tensor_copy(at_t.rearrange("p a b -> p (a b)"), pat)

                    hglob = g * 4 + hh
                    for t in range(4):
                        mm(
                            py[hh * 32 : (hh + 1) * 32, :],
                            vn_sb[:, t, hglob * 32 : (hglob + 1) * 32],
                            at_t[:, t, :],
                            t == 0, t == 3,
                            tile_position=(0, hh * 32),
                        )
                nc.scalar.copy(yt_sb[:, g, :], py)

            for cc in range(2):
                po = ps_s.tile([128, IB], F32)
                for k in range(2):
                    mm(po, wchunk("Wo", k, cc), yt_sb[:, k, :], k == 0, k == 1)
                nc.scalar.activation(
                    outt_sb[:, cc, :], po, mybir.ActivationFunctionType.Identity,
                    bias=b_sb["bo"][:, cc : cc + 1],
                )

            nc.sync.dma_start(
                out=out_d.rearrange("(a p) n -> p a n", p=128), in_=outt_sb
            )

    if compile:
        nc.compile()
    return nc


_cached_nc = None
last_exec_time_ns = None


def _get_nc():
    global _cached_nc
    if _cached_nc is None:
        _cached_nc = build()
    return _cached_nc


def kernel(X, E, Wq, bq, Wk, bk, Wv, bv, Wo, bo, We1, be1, We2, be2, **kw):
    X = np.asarray(X, np.float32)
    E = np.asarray(E, np.float32)

    nc = _get_nc()

    shared = {
        "Wq": np.ascontiguousarray(Wq, np.float32),
        "Wk": np.ascontiguousarray(Wk, np.float32),
        "Wv": np.ascontiguousarray(Wv, np.float32),
        "Wo": np.ascontiguousarray(Wo, np.float32),
        "We1": np.ascontiguousarray(We1, np.float32),
        "We2": np.ascontiguousarray(We2, np.float32).reshape(D, 1),
        "bq": np.ascontiguousarray(bq, np.float32),
        "be1": np.ascontiguousarray(be1, np.float32),
        "bv": np.ascontiguousarray(bv, np.float32),
        "bo": np.ascontiguousarray(bo, np.float32),
    }
    in_maps = []
    for c in range(NCORES):
        b, blk = c // 4, c % 4
        i0 = blk * IB
        m = dict(shared)
        m["E"] = np.ascontiguousarray(E[b, i0 : i0 + IB])
        m["XT"] = np.ascontiguousarray(X[b].T)
        m["XTQ"] = np.ascontiguousarray(X[b, i0 : i0 + IB].T)
        in_maps.append(m)

    trace = bool(int(os.environ.get("KERNEL_PROFILE", "0")))
    res = run_bass_kernel_spmd(nc, in_maps, list(range(NCORES)), trace=trace)
    global last_exec_time_ns
    last_exec_time_ns = res.exec_time_ns

    out = np.zeros((B, N, D), np.float32)
    for c in range(NCORES):
        b, blk = c // 4, c % 4
        i0 = blk * IB
        out[b, i0 : i0 + IB, :] = res.results[c]["OUT_T"].T
    return out
